# revision 29
# baseline (speedup 1.0000x reference)
"""AUGRU cell (attention-scaled GRU update) on 8 Trainium2 NeuronCores.

Data-parallel: batch B=65536 sharded 8 ways (8192 rows/core); gate weights
replicated.  Per core:

  gates_x = x @ W_x.T + b_x          (8192,384)
  gates_h = h @ W_h.T + b_h
  u = sigmoid(.. u block ..); r = sigmoid(.. r block ..)
  h_tilde = tanh(x_c + r * h_c)
  h_new = h_prev + att*u*(h_tilde - h_prev)

v8 design — gate-major layout, host-staged transposed operands:
  - each core receives xT/hT = x/h shard transposed to [I, rows] (a host
    layout/sharding choice; the contraction needs I on partitions either
    way) and the output is produced transposed, flipped back on the host.
  - gates live in PSUM as [gate_type][128, 512]: U/R/Cx/Ch banks.  Two
    accumulating fp32r matmuls for U and R, one each for Cx/Ch; weights
    transposed once at setup on the PE and kept fp32r (fp32-rate 1 cyc/row
    at N>=256, fp32-class accuracy).
  - biases are per-partition in this layout: sigmoid takes them via the
    ACT bias operand; the candidate path folds them into the two DVE
    scalar_tensor_tensor ops.  No bias matmuls, no device transposes,
    no PSUM round-trip copies, no casts on the matmul path.
  - epilogue: u/r/tanh outputs bf16; blend (t-h)*u*att in packed bf16 DVE
    ops; d and the final add against fp32 hT on GPSIMD keep h_prev exact.
"""

import sys

sys.path.insert(0, "/opt/trn_rl_repo")

import os
from contextlib import ExitStack

import numpy as np

import concourse.bass as bass
import concourse.tile as tile
from concourse import bacc, mybir
from concourse.bass_utils import run_bass_kernel_spmd
from concourse.masks import make_identity

F32 = mybir.dt.float32
F32R = mybir.dt.float32r
BF16 = mybir.dt.bfloat16
AF = mybir.ActivationFunctionType
OP = mybir.AluOpType

B = 65536
NCORES = 8
BL = int(os.environ.get("AUGRU_BL", B // NCORES))  # 8192 rows per core
I = 128
H = 128
G3 = 3 * H
P = 128
ROWS = 512  # batch rows per group (one fp32 PSUM bank per gate type)
NGROUPS = BL // ROWS

# PSUM banks per group: 0 = U, 1 = R, 2 = Cx, 3 = Ch   (each [128, 512])


def build_program():
    nc = bacc.Bacc("TRN2", target_bir_lowering=False, debug=False)

    xT_d = nc.dram_tensor("xT", [I, BL], F32, kind="ExternalInput").ap()
    hT_d = nc.dram_tensor("hT", [H, BL], F32, kind="ExternalInput").ap()
    a_d = nc.dram_tensor("att_score", [BL], F32, kind="ExternalInput").ap()
    wx_d = nc.dram_tensor("W_x", [G3, I], F32, kind="ExternalInput").ap()
    bx_d = nc.dram_tensor("b_x", [G3], F32, kind="ExternalInput").ap()
    wh_d = nc.dram_tensor("W_h", [G3, H], F32, kind="ExternalInput").ap()
    bh_d = nc.dram_tensor("b_h", [G3], F32, kind="ExternalInput").ap()
    o_d = nc.dram_tensor("h_newT", [H, BL], F32, kind="ExternalOutput").ap()

    with tile.TileContext(nc) as tc, ExitStack() as ctx:
        consts = ctx.enter_context(tc.tile_pool(name="consts", bufs=1))
        io = ctx.enter_context(tc.tile_pool(name="io", bufs=8))
        ep = ctx.enter_context(tc.tile_pool(name="ep", bufs=6))
        pg = ctx.enter_context(tc.tile_pool(name="pg", bufs=2, space="PSUM"))

        # ---------------- one-time setup ----------------
        ident = consts.tile([P, P], F32)
        make_identity(nc, ident)

        # weight blocks [128 gate, 128 I] -> transposed [128 I, 128 gate],
        # rounded to fp32r for full-rate fp32 matmuls
        wxn = consts.tile([P, 3, I], F32, tag="wxn")
        nc.sync.dma_start(wxn, wx_d.rearrange("(b g) i -> g b i", g=P))
        whn = consts.tile([P, 3, I], F32, tag="whn")
        nc.sync.dma_start(whn, wh_d.rearrange("(b g) i -> g b i", g=P))
        wT = consts.tile([P, 6, P], F32R, tag="wT")  # [xu, xr, xc, hu, hr, hc]
        for j in range(3):
            ps = pg.tile([P, 4, ROWS], F32, tag="g")
            nc.tensor.matmul(ps[:, 0, 0:P], lhsT=wxn[:, j, :], rhs=ident,
                             is_transpose=True)
            nc.vector.tensor_copy(wT[:, j, :], ps[:, 0, 0:P])
        for j in range(3):
            ps = pg.tile([P, 4, ROWS], F32, tag="g")
            nc.tensor.matmul(ps[:, 0, 0:P], lhsT=whn[:, j, :], rhs=ident,
                             is_transpose=True)
            nc.vector.tensor_copy(wT[:, 3 + j, :], ps[:, 0, 0:P])

        # per-partition bias columns [128, 1]: b_u+b_hu | b_r+b_hr | b_xc | b_hc
        bxc = consts.tile([P, 3], F32, tag="bxc")
        nc.sync.dma_start(bxc, bx_d.rearrange("(b p) -> p b", p=P))
        bhc = consts.tile([P, 3], F32, tag="bhc")
        nc.sync.dma_start(bhc, bh_d.rearrange("(b p) -> p b", p=P))
        bcol = consts.tile([P, 4], F32, tag="bcol")
        nc.vector.tensor_tensor(bcol[:, 0:2], bxc[:, 0:2], bhc[:, 0:2], OP.add)
        nc.vector.tensor_copy(bcol[:, 2:3], bxc[:, 2:3])
        nc.vector.tensor_copy(bcol[:, 3:4], bhc[:, 2:3])

        # att broadcast to all partitions (GPSIMD), then cast to bf16
        att1 = consts.tile([1, BL], F32R, tag="att1")
        nc.sync.dma_start(att1, a_d.unsqueeze(0).bitcast(F32R))
        ones_f = consts.tile([1, P], F32, tag="ones_f")
        nc.vector.memset(ones_f, 1.0)
        ones = consts.tile([1, P], F32R, tag="ones")
        nc.vector.tensor_copy(ones, ones_f)
        attb = consts.tile([P, BL], BF16, tag="attb")

        # ---------------- pipelined main loop ----------------
        stA = [None] * (NGROUPS + 4)
        stB = [None] * (NGROUPS + 4)
        stC = [None] * (NGROUPS + 4)

        def stage_a(g):
            b0 = g * ROWS
            xs = io.tile([P, ROWS], F32R, tag="xs")
            nc.sync.dma_start(xs, xT_d[:, b0 : b0 + ROWS].bitcast(F32R))
            hs = io.tile([P, ROWS], F32R, tag="hs")
            nc.sync.dma_start(hs, hT_d[:, b0 : b0 + ROWS].bitcast(F32R))
            return xs, hs

        def stage_b(g):
            xs, hs = stA[g]
            xr = xs
            hr = hs
            gp = pg.tile([P, 4, ROWS], F32, tag="g")
            nc.tensor.matmul(gp[:, 0, :], lhsT=wT[:, 0, :], rhs=xr, start=True, stop=False)
            nc.tensor.matmul(gp[:, 1, :], lhsT=wT[:, 1, :], rhs=xr, start=True, stop=False)
            nc.tensor.matmul(gp[:, 2, :], lhsT=wT[:, 2, :], rhs=xr, start=True, stop=True)
            nc.tensor.matmul(gp[:, 3, :], lhsT=wT[:, 5, :], rhs=hr, start=True, stop=True)
            nc.tensor.matmul(gp[:, 0, :], lhsT=wT[:, 3, :], rhs=hr, start=False, stop=True)
            nc.tensor.matmul(gp[:, 1, :], lhsT=wT[:, 4, :], rhs=hr, start=False, stop=True)
            return gp

        def stage_c(g):
            xs, hs = stA[g]
            gp = stB[g]
            r = ep.tile([P, ROWS], BF16, tag="r")
            nc.scalar.activation(r, gp[:, 1, :], AF.Sigmoid, bias=bcol[:, 1:2])
            u = ep.tile([P, ROWS], BF16, tag="u")
            nc.scalar.activation(u, gp[:, 0, :], AF.Sigmoid, bias=bcol[:, 0:1])
            m = ep.tile([P, ROWS], F32, tag="m")
            nc.vector.scalar_tensor_tensor(
                m, in0=gp[:, 3, :], scalar=bcol[:, 3:4], in1=r,
                op0=OP.add, op1=OP.mult,
            )
            pre = ep.tile([P, ROWS], F32, tag="pre")
            nc.vector.scalar_tensor_tensor(
                pre, in0=gp[:, 2, :], scalar=bcol[:, 2:3], in1=m,
                op0=OP.add, op1=OP.add,
            )
            # att broadcast chunk for this group, into the Cx bank that
            # `pre` just drained (ones is K=1, so this is a cheap matmul)
            nc.tensor.matmul(
                gp[:, 2, :], lhsT=ones,
                rhs=att1[:, g * ROWS : (g + 1) * ROWS],
                start=True, stop=True, skip_group_check=True,
            )
            nc.scalar.copy(attb[:, g * ROWS : (g + 1) * ROWS], gp[:, 2, :])
            tb = ep.tile([P, ROWS], BF16, tag="tb")
            nc.scalar.activation(tb, pre, AF.Tanh)
            d = ep.tile([P, ROWS], BF16, tag="d")
            nc.gpsimd.tensor_tensor(d, tb, hs.bitcast(F32), OP.subtract)
            e1 = ep.tile([P, ROWS], BF16, tag="e1")
            nc.vector.tensor_tensor(e1, d, u, OP.mult)
            e2 = ep.tile([P, ROWS], BF16, tag="e2")
            nc.vector.tensor_tensor(
                e2, e1, attb[:, g * ROWS : (g + 1) * ROWS], OP.mult
            )
            return e2

        def stage_c2(g):
            b0 = g * ROWS
            xs, hs = stA[g]
            e2 = stC[g]
            ho = ep.tile([P, ROWS], F32, tag="ho")
            nc.gpsimd.tensor_tensor(ho, e2, hs.bitcast(F32), OP.add)
            # store from the GPSIMD queue: it directly follows ho there, so
            # the sync queue (loads) never head-of-line blocks on epilogues
            nc.gpsimd.dma_start(o_d[:, b0 : b0 + ROWS], ho)

        for k in range(NGROUPS + 4):
            if k < NGROUPS:
                stA[k] = stage_a(k)
            if 2 <= k < NGROUPS + 2:
                stB[k - 2] = stage_b(k - 2)
            if k >= 4:
                stage_c2(k - 4)
            if 3 <= k < NGROUPS + 3:
                stC[k - 3] = stage_c(k - 3)

    nc.compile()
    return nc


_NC_CACHE = []


def _get_nc():
    if not _NC_CACHE:
        _NC_CACHE.append(build_program())
    return _NC_CACHE[0]


def kernel(x, h_prev, att_score, W_x, b_x, W_h, b_h, **_unused):
    x = np.asarray(x, dtype=np.float32)
    h_prev = np.asarray(h_prev, dtype=np.float32)
    att_score = np.ascontiguousarray(np.asarray(att_score, dtype=np.float32))
    W_x = np.ascontiguousarray(np.asarray(W_x, dtype=np.float32))
    b_x = np.ascontiguousarray(np.asarray(b_x, dtype=np.float32))
    W_h = np.ascontiguousarray(np.asarray(W_h, dtype=np.float32))
    b_h = np.ascontiguousarray(np.asarray(b_h, dtype=np.float32))

    nc = _get_nc()
    in_maps = []
    for c in range(NCORES):
        s = slice(c * BL, (c + 1) * BL)
        in_maps.append(
            {
                "xT": np.ascontiguousarray(x[s].T),
                "hT": np.ascontiguousarray(h_prev[s].T),
                "att_score": np.ascontiguousarray(att_score[s]),
                "W_x": W_x,
                "b_x": b_x,
                "W_h": W_h,
                "b_h": b_h,
            }
        )
    res = run_bass_kernel_spmd(nc, in_maps, list(range(NCORES)))
    out = np.concatenate(
        [np.ascontiguousarray(res.results[c]["h_newT"].T) for c in range(NCORES)],
        axis=0,
    )
    return out


# revision 30
# speedup vs baseline: 1.1741x; 1.1741x over previous
"""AUGRU cell (attention-scaled GRU update) on 8 Trainium2 NeuronCores.

Data-parallel: batch B=65536 sharded 8 ways (8192 rows/core); gate weights
replicated.  Per core:

  gates_x = x @ W_x.T + b_x          (8192,384)
  gates_h = h @ W_h.T + b_h
  u = sigmoid(.. u block ..); r = sigmoid(.. r block ..)
  h_tilde = tanh(x_c + r * h_c)
  h_new = h_prev + att*u*(h_tilde - h_prev)

v8 design — gate-major layout, host-staged transposed operands:
  - each core receives xT/hT = x/h shard transposed to [I, rows] (a host
    layout/sharding choice; the contraction needs I on partitions either
    way) and the output is produced transposed, flipped back on the host.
  - gates live in PSUM as [gate_type][128, 512]: U/R/Cx/Ch banks.  Two
    accumulating fp32r matmuls for U and R, one each for Cx/Ch; weights
    transposed once at setup on the PE and kept fp32r (fp32-rate 1 cyc/row
    at N>=256, fp32-class accuracy).
  - biases are per-partition in this layout: sigmoid takes them via the
    ACT bias operand; the candidate path folds them into the two DVE
    scalar_tensor_tensor ops.  No bias matmuls, no device transposes,
    no PSUM round-trip copies, no casts on the matmul path.
  - epilogue: u/r/tanh outputs bf16; blend (t-h)*u*att in packed bf16 DVE
    ops; d and the final add against fp32 hT on GPSIMD keep h_prev exact.
"""

import sys

sys.path.insert(0, "/opt/trn_rl_repo")

import os
from contextlib import ExitStack

import numpy as np

import concourse.bass as bass
import concourse.tile as tile
from concourse import bacc, mybir
from concourse.bass_utils import run_bass_kernel_spmd
from concourse.masks import make_identity

F32 = mybir.dt.float32
F32R = mybir.dt.float32r
BF16 = mybir.dt.bfloat16
AF = mybir.ActivationFunctionType
OP = mybir.AluOpType

B = 65536
NCORES = 8
BL = int(os.environ.get("AUGRU_BL", B // NCORES))  # 8192 rows per core
I = 128
H = 128
G3 = 3 * H
P = 128
ROWS = 512  # batch rows per group (one fp32 PSUM bank per gate type)
NGROUPS = BL // ROWS

# PSUM banks per group: 0 = U, 1 = R, 2 = Cx, 3 = Ch   (each [128, 512])


def build_program():
    nc = bacc.Bacc("TRN2", target_bir_lowering=False, debug=False)

    xT_d = nc.dram_tensor("xT", [I, BL], F32, kind="ExternalInput").ap()
    hT_d = nc.dram_tensor("hT", [H, BL], F32, kind="ExternalInput").ap()
    a_d = nc.dram_tensor("att_score", [BL], F32, kind="ExternalInput").ap()
    wx_d = nc.dram_tensor("W_x", [G3, I], F32, kind="ExternalInput").ap()
    bx_d = nc.dram_tensor("b_x", [G3], F32, kind="ExternalInput").ap()
    wh_d = nc.dram_tensor("W_h", [G3, H], F32, kind="ExternalInput").ap()
    bh_d = nc.dram_tensor("b_h", [G3], F32, kind="ExternalInput").ap()
    o_d = nc.dram_tensor("h_newT", [H, BL], F32, kind="ExternalOutput").ap()

    with tile.TileContext(nc) as tc, ExitStack() as ctx:
        consts = ctx.enter_context(tc.tile_pool(name="consts", bufs=1))
        io = ctx.enter_context(tc.tile_pool(name="io", bufs=8))
        ep = ctx.enter_context(tc.tile_pool(name="ep", bufs=6))
        pg = ctx.enter_context(tc.tile_pool(name="pg", bufs=2, space="PSUM"))

        # ---------------- one-time setup ----------------
        ident = consts.tile([P, P], F32)
        make_identity(nc, ident)

        # weight blocks [128 gate, 128 I] -> transposed [128 I, 128 gate],
        # rounded to fp32r for full-rate fp32 matmuls
        wxn = consts.tile([P, 3, I], F32, tag="wxn")
        nc.sync.dma_start(wxn, wx_d.rearrange("(b g) i -> g b i", g=P))
        whn = consts.tile([P, 3, I], F32, tag="whn")
        nc.sync.dma_start(whn, wh_d.rearrange("(b g) i -> g b i", g=P))
        wT = consts.tile([P, 6, P], F32R, tag="wT")  # [xu, xr, xc, hu, hr, hc]
        for j in range(3):
            ps = pg.tile([P, 4, ROWS], F32, tag="g")
            nc.tensor.matmul(ps[:, 0, 0:P], lhsT=wxn[:, j, :], rhs=ident,
                             is_transpose=True)
            nc.vector.tensor_copy(wT[:, j, :], ps[:, 0, 0:P])
        for j in range(3):
            ps = pg.tile([P, 4, ROWS], F32, tag="g")
            nc.tensor.matmul(ps[:, 0, 0:P], lhsT=whn[:, j, :], rhs=ident,
                             is_transpose=True)
            nc.vector.tensor_copy(wT[:, 3 + j, :], ps[:, 0, 0:P])

        # per-partition bias columns [128, 1]: b_u+b_hu | b_r+b_hr | b_xc | b_hc
        bxc = consts.tile([P, 3], F32, tag="bxc")
        nc.sync.dma_start(bxc, bx_d.rearrange("(b p) -> p b", p=P))
        bhc = consts.tile([P, 3], F32, tag="bhc")
        nc.sync.dma_start(bhc, bh_d.rearrange("(b p) -> p b", p=P))
        bcol = consts.tile([P, 4], F32, tag="bcol")
        nc.vector.tensor_tensor(bcol[:, 0:2], bxc[:, 0:2], bhc[:, 0:2], OP.add)
        nc.vector.tensor_copy(bcol[:, 2:3], bxc[:, 2:3])
        nc.vector.tensor_copy(bcol[:, 3:4], bhc[:, 2:3])

        # att broadcast to all partitions (GPSIMD), then cast to bf16
        att1 = consts.tile([1, BL], F32R, tag="att1")
        nc.sync.dma_start(att1, a_d.unsqueeze(0).bitcast(F32R))
        ones_f = consts.tile([1, P], F32, tag="ones_f")
        nc.vector.memset(ones_f, 1.0)
        ones = consts.tile([1, P], F32R, tag="ones")
        nc.vector.tensor_copy(ones, ones_f)
        attb = consts.tile([P, BL], BF16, tag="attb")
        for gch in range(NGROUPS):
            ps = pg.tile([P, 4, ROWS], F32, tag="g")
            nc.tensor.matmul(
                ps[:, 0, :], lhsT=ones,
                rhs=att1[:, gch * ROWS : (gch + 1) * ROWS],
                start=True, stop=True,
            )
            nc.scalar.copy(attb[:, gch * ROWS : (gch + 1) * ROWS], ps[:, 0, :])

        # ---------------- pipelined main loop ----------------
        stA = [None] * (NGROUPS + 4)
        stB = [None] * (NGROUPS + 4)
        stC = [None] * (NGROUPS + 4)

        def stage_a(g):
            b0 = g * ROWS
            xs = io.tile([P, ROWS], F32R, tag="xs")
            nc.sync.dma_start(xs, xT_d[:, b0 : b0 + ROWS].bitcast(F32R))
            hs = io.tile([P, ROWS], F32R, tag="hs")
            nc.sync.dma_start(hs, hT_d[:, b0 : b0 + ROWS].bitcast(F32R))
            return xs, hs

        def stage_b(g):
            xs, hs = stA[g]
            xr = xs
            hr = hs
            gp = pg.tile([P, 4, ROWS], F32, tag="g")
            nc.tensor.matmul(gp[:, 0, :], lhsT=wT[:, 0, :], rhs=xr, start=True, stop=False)
            nc.tensor.matmul(gp[:, 1, :], lhsT=wT[:, 1, :], rhs=xr, start=True, stop=False)
            nc.tensor.matmul(gp[:, 2, :], lhsT=wT[:, 2, :], rhs=xr, start=True, stop=True)
            nc.tensor.matmul(gp[:, 3, :], lhsT=wT[:, 5, :], rhs=hr, start=True, stop=True)
            nc.tensor.matmul(gp[:, 0, :], lhsT=wT[:, 3, :], rhs=hr, start=False, stop=True)
            nc.tensor.matmul(gp[:, 1, :], lhsT=wT[:, 4, :], rhs=hr, start=False, stop=True)
            return gp

        def stage_c(g):
            xs, hs = stA[g]
            gp = stB[g]
            r = ep.tile([P, ROWS], BF16, tag="r")
            nc.scalar.activation(r, gp[:, 1, :], AF.Sigmoid, bias=bcol[:, 1:2])
            u = ep.tile([P, ROWS], BF16, tag="u")
            nc.scalar.activation(u, gp[:, 0, :], AF.Sigmoid, bias=bcol[:, 0:1])
            m = ep.tile([P, ROWS], F32, tag="m")
            nc.vector.scalar_tensor_tensor(
                m, in0=gp[:, 3, :], scalar=bcol[:, 3:4], in1=r,
                op0=OP.add, op1=OP.mult,
            )
            pre = ep.tile([P, ROWS], F32, tag="pre")
            nc.vector.scalar_tensor_tensor(
                pre, in0=gp[:, 2, :], scalar=bcol[:, 2:3], in1=m,
                op0=OP.add, op1=OP.add,
            )
            tb = ep.tile([P, ROWS], BF16, tag="tb")
            nc.scalar.activation(tb, pre, AF.Tanh)
            d = ep.tile([P, ROWS], BF16, tag="d")
            nc.gpsimd.tensor_tensor(d, tb, hs.bitcast(F32), OP.subtract)
            e1 = ep.tile([P, ROWS], BF16, tag="e1")
            nc.vector.tensor_tensor(e1, d, u, OP.mult)
            e2 = ep.tile([P, ROWS], BF16, tag="e2")
            nc.vector.tensor_tensor(
                e2, e1, attb[:, g * ROWS : (g + 1) * ROWS], OP.mult
            )
            return e2

        def stage_c2(g):
            b0 = g * ROWS
            xs, hs = stA[g]
            e2 = stC[g]
            ho = ep.tile([P, ROWS], F32, tag="ho")
            nc.gpsimd.tensor_tensor(ho, e2, hs.bitcast(F32), OP.add)
            # store from the GPSIMD queue: it directly follows ho there, so
            # the sync queue (loads) never head-of-line blocks on epilogues
            nc.gpsimd.dma_start(o_d[:, b0 : b0 + ROWS], ho)

        for k in range(NGROUPS + 4):
            if k < NGROUPS:
                stA[k] = stage_a(k)
            if 2 <= k < NGROUPS + 2:
                stB[k - 2] = stage_b(k - 2)
            if k >= 4:
                stage_c2(k - 4)
            if 3 <= k < NGROUPS + 3:
                stC[k - 3] = stage_c(k - 3)

    nc.compile()
    return nc


_NC_CACHE = []


def _get_nc():
    if not _NC_CACHE:
        _NC_CACHE.append(build_program())
    return _NC_CACHE[0]


def kernel(x, h_prev, att_score, W_x, b_x, W_h, b_h, **_unused):
    x = np.asarray(x, dtype=np.float32)
    h_prev = np.asarray(h_prev, dtype=np.float32)
    att_score = np.ascontiguousarray(np.asarray(att_score, dtype=np.float32))
    W_x = np.ascontiguousarray(np.asarray(W_x, dtype=np.float32))
    b_x = np.ascontiguousarray(np.asarray(b_x, dtype=np.float32))
    W_h = np.ascontiguousarray(np.asarray(W_h, dtype=np.float32))
    b_h = np.ascontiguousarray(np.asarray(b_h, dtype=np.float32))

    nc = _get_nc()
    in_maps = []
    for c in range(NCORES):
        s = slice(c * BL, (c + 1) * BL)
        in_maps.append(
            {
                "xT": np.ascontiguousarray(x[s].T),
                "hT": np.ascontiguousarray(h_prev[s].T),
                "att_score": np.ascontiguousarray(att_score[s]),
                "W_x": W_x,
                "b_x": b_x,
                "W_h": W_h,
                "b_h": b_h,
            }
        )
    res = run_bass_kernel_spmd(nc, in_maps, list(range(NCORES)))
    out = np.concatenate(
        [np.ascontiguousarray(res.results[c]["h_newT"].T) for c in range(NCORES)],
        axis=0,
    )
    return out


# revision 31
# speedup vs baseline: 1.1879x; 1.0118x over previous
"""AUGRU cell (attention-scaled GRU update) on 8 Trainium2 NeuronCores.

Data-parallel: batch B=65536 sharded 8 ways (8192 rows/core); gate weights
replicated.  Per core:

  gates_x = x @ W_x.T + b_x          (8192,384)
  gates_h = h @ W_h.T + b_h
  u = sigmoid(.. u block ..); r = sigmoid(.. r block ..)
  h_tilde = tanh(x_c + r * h_c)
  h_new = h_prev + att*u*(h_tilde - h_prev)

v8 design — gate-major layout, host-staged transposed operands:
  - each core receives xT/hT = x/h shard transposed to [I, rows] (a host
    layout/sharding choice; the contraction needs I on partitions either
    way) and the output is produced transposed, flipped back on the host.
  - gates live in PSUM as [gate_type][128, 512]: U/R/Cx/Ch banks.  Two
    accumulating fp32r matmuls for U and R, one each for Cx/Ch; weights
    transposed once at setup on the PE and kept fp32r (fp32-rate 1 cyc/row
    at N>=256, fp32-class accuracy).
  - biases are per-partition in this layout: sigmoid takes them via the
    ACT bias operand; the candidate path folds them into the two DVE
    scalar_tensor_tensor ops.  No bias matmuls, no device transposes,
    no PSUM round-trip copies, no casts on the matmul path.
  - epilogue: u/r/tanh outputs bf16; blend (t-h)*u*att in packed bf16 DVE
    ops; d and the final add against fp32 hT on GPSIMD keep h_prev exact.
"""

import sys

sys.path.insert(0, "/opt/trn_rl_repo")

import os
from contextlib import ExitStack

import numpy as np

import concourse.bass as bass
import concourse.tile as tile
from concourse import bacc, mybir
from concourse.bass_utils import run_bass_kernel_spmd
from concourse.masks import make_identity

F32 = mybir.dt.float32
F32R = mybir.dt.float32r
BF16 = mybir.dt.bfloat16
AF = mybir.ActivationFunctionType
OP = mybir.AluOpType

B = 65536
NCORES = 8
BL = int(os.environ.get("AUGRU_BL", B // NCORES))  # 8192 rows per core
I = 128
H = 128
G3 = 3 * H
P = 128
ROWS = 512  # batch rows per group (one fp32 PSUM bank per gate type)
NGROUPS = BL // ROWS

# PSUM banks per group: 0 = U, 1 = R, 2 = Cx, 3 = Ch   (each [128, 512])


def build_program():
    nc = bacc.Bacc("TRN2", target_bir_lowering=False, debug=False)

    xT_d = nc.dram_tensor("xT", [I, BL], F32, kind="ExternalInput").ap()
    hT_d = nc.dram_tensor("hT", [H, BL], F32, kind="ExternalInput").ap()
    a_d = nc.dram_tensor("att_score", [BL], F32, kind="ExternalInput").ap()
    wx_d = nc.dram_tensor("W_x", [G3, I], F32, kind="ExternalInput").ap()
    bx_d = nc.dram_tensor("b_x", [G3], F32, kind="ExternalInput").ap()
    wh_d = nc.dram_tensor("W_h", [G3, H], F32, kind="ExternalInput").ap()
    bh_d = nc.dram_tensor("b_h", [G3], F32, kind="ExternalInput").ap()
    o_d = nc.dram_tensor("h_newT", [H, BL], F32, kind="ExternalOutput").ap()

    with tile.TileContext(nc) as tc, ExitStack() as ctx:
        consts = ctx.enter_context(tc.tile_pool(name="consts", bufs=1))
        io = ctx.enter_context(tc.tile_pool(name="io", bufs=8))
        ep = ctx.enter_context(tc.tile_pool(name="ep", bufs=6))
        pg = ctx.enter_context(tc.tile_pool(name="pg", bufs=2, space="PSUM"))

        # ---------------- one-time setup ----------------
        ident = consts.tile([P, P], F32)
        make_identity(nc, ident)

        # weight blocks [128 gate, 128 I] -> transposed [128 I, 128 gate],
        # rounded to fp32r for full-rate fp32 matmuls
        wxn = consts.tile([P, 3, I], F32, tag="wxn")
        nc.sync.dma_start(wxn, wx_d.rearrange("(b g) i -> g b i", g=P))
        whn = consts.tile([P, 3, I], F32, tag="whn")
        nc.sync.dma_start(whn, wh_d.rearrange("(b g) i -> g b i", g=P))
        wT = consts.tile([P, 6, P], F32R, tag="wT")  # [xu, xr, xc, hu, hr, hc]
        for j in range(3):
            ps = pg.tile([P, ROWS], F32, tag="gU")
            nc.tensor.matmul(ps[:, 0:P], lhsT=wxn[:, j, :], rhs=ident,
                             is_transpose=True)
            nc.vector.tensor_copy(wT[:, j, :], ps[:, 0:P])
        for j in range(3):
            ps = pg.tile([P, ROWS], F32, tag="gU")
            nc.tensor.matmul(ps[:, 0:P], lhsT=whn[:, j, :], rhs=ident,
                             is_transpose=True)
            nc.vector.tensor_copy(wT[:, 3 + j, :], ps[:, 0:P])

        # per-partition bias columns [128, 1]: b_u+b_hu | b_r+b_hr | b_xc | b_hc
        bxc = consts.tile([P, 3], F32, tag="bxc")
        nc.sync.dma_start(bxc, bx_d.rearrange("(b p) -> p b", p=P))
        bhc = consts.tile([P, 3], F32, tag="bhc")
        nc.sync.dma_start(bhc, bh_d.rearrange("(b p) -> p b", p=P))
        bcol = consts.tile([P, 4], F32, tag="bcol")
        nc.vector.tensor_tensor(bcol[:, 0:2], bxc[:, 0:2], bhc[:, 0:2], OP.add)
        nc.vector.tensor_copy(bcol[:, 2:3], bxc[:, 2:3])
        nc.vector.tensor_copy(bcol[:, 3:4], bhc[:, 2:3])

        # att broadcast to all partitions (GPSIMD), then cast to bf16
        att1 = consts.tile([1, BL], F32R, tag="att1")
        nc.sync.dma_start(att1, a_d.unsqueeze(0).bitcast(F32R))
        ones_f = consts.tile([1, P], F32, tag="ones_f")
        nc.vector.memset(ones_f, 1.0)
        ones = consts.tile([1, P], F32R, tag="ones")
        nc.vector.tensor_copy(ones, ones_f)
        attb = consts.tile([P, BL], BF16, tag="attb")
        for gch in range(NGROUPS):
            ps = pg.tile([P, ROWS], F32, tag="gR")
            nc.tensor.matmul(
                ps[:, :], lhsT=ones,
                rhs=att1[:, gch * ROWS : (gch + 1) * ROWS],
                start=True, stop=True,
            )
            nc.scalar.copy(attb[:, gch * ROWS : (gch + 1) * ROWS], ps[:, :])

        # ---------------- pipelined main loop ----------------
        stA = [None] * (NGROUPS + 4)
        stB = [None] * (NGROUPS + 4)
        stC = [None] * (NGROUPS + 4)

        def stage_a(g):
            b0 = g * ROWS
            xs = io.tile([P, ROWS], F32R, tag="xs")
            nc.sync.dma_start(xs, xT_d[:, b0 : b0 + ROWS].bitcast(F32R))
            hs = io.tile([P, ROWS], F32R, tag="hs")
            nc.sync.dma_start(hs, hT_d[:, b0 : b0 + ROWS].bitcast(F32R))
            return xs, hs

        def stage_b(g):
            xs, hs = stA[g]
            xr = xs
            hr = hs
            gU = pg.tile([P, ROWS], F32, tag="gU")
            gR = pg.tile([P, ROWS], F32, tag="gR")
            gCx = pg.tile([P, ROWS], F32, tag="gCx")
            gCh = pg.tile([P, ROWS], F32, tag="gCh")
            nc.tensor.matmul(gU, lhsT=wT[:, 0, :], rhs=xr, start=True, stop=False)
            nc.tensor.matmul(gR, lhsT=wT[:, 1, :], rhs=xr, start=True, stop=False)
            nc.tensor.matmul(gCx, lhsT=wT[:, 2, :], rhs=xr, start=True, stop=True)
            nc.tensor.matmul(gCh, lhsT=wT[:, 5, :], rhs=hr, start=True, stop=True)
            nc.tensor.matmul(gU, lhsT=wT[:, 3, :], rhs=hr, start=False, stop=True)
            nc.tensor.matmul(gR, lhsT=wT[:, 4, :], rhs=hr, start=False, stop=True)
            return gU, gR, gCx, gCh

        def stage_c(g):
            xs, hs = stA[g]
            gU, gR, gCx, gCh = stB[g]
            r = ep.tile([P, ROWS], BF16, tag="r")
            nc.scalar.activation(r, gR, AF.Sigmoid, bias=bcol[:, 1:2])
            u = ep.tile([P, ROWS], BF16, tag="u")
            nc.scalar.activation(u, gU, AF.Sigmoid, bias=bcol[:, 0:1])
            m = ep.tile([P, ROWS], F32, tag="m")
            nc.vector.scalar_tensor_tensor(
                m, in0=gCh, scalar=bcol[:, 3:4], in1=r,
                op0=OP.add, op1=OP.mult,
            )
            pre = ep.tile([P, ROWS], F32, tag="pre")
            nc.vector.scalar_tensor_tensor(
                pre, in0=gCx, scalar=bcol[:, 2:3], in1=m,
                op0=OP.add, op1=OP.add,
            )
            tb = ep.tile([P, ROWS], BF16, tag="tb")
            nc.scalar.activation(tb, pre, AF.Tanh)
            d = ep.tile([P, ROWS], BF16, tag="d")
            nc.gpsimd.tensor_tensor(d, tb, hs.bitcast(F32), OP.subtract)
            e1 = ep.tile([P, ROWS], BF16, tag="e1")
            nc.vector.tensor_tensor(e1, d, u, OP.mult)
            e2 = ep.tile([P, ROWS], BF16, tag="e2")
            nc.vector.tensor_tensor(
                e2, e1, attb[:, g * ROWS : (g + 1) * ROWS], OP.mult
            )
            return e2

        def stage_c2(g):
            b0 = g * ROWS
            xs, hs = stA[g]
            e2 = stC[g]
            ho = ep.tile([P, ROWS], F32, tag="ho")
            nc.gpsimd.tensor_tensor(ho, e2, hs.bitcast(F32), OP.add)
            # store from the GPSIMD queue: it directly follows ho there, so
            # the sync queue (loads) never head-of-line blocks on epilogues
            nc.gpsimd.dma_start(o_d[:, b0 : b0 + ROWS], ho)

        for k in range(NGROUPS + 4):
            if k < NGROUPS:
                stA[k] = stage_a(k)
            if 2 <= k < NGROUPS + 2:
                stB[k - 2] = stage_b(k - 2)
            if k >= 4:
                stage_c2(k - 4)
            if 3 <= k < NGROUPS + 3:
                stC[k - 3] = stage_c(k - 3)

    nc.compile()
    return nc


_NC_CACHE = []


def _get_nc():
    if not _NC_CACHE:
        _NC_CACHE.append(build_program())
    return _NC_CACHE[0]


def kernel(x, h_prev, att_score, W_x, b_x, W_h, b_h, **_unused):
    x = np.asarray(x, dtype=np.float32)
    h_prev = np.asarray(h_prev, dtype=np.float32)
    att_score = np.ascontiguousarray(np.asarray(att_score, dtype=np.float32))
    W_x = np.ascontiguousarray(np.asarray(W_x, dtype=np.float32))
    b_x = np.ascontiguousarray(np.asarray(b_x, dtype=np.float32))
    W_h = np.ascontiguousarray(np.asarray(W_h, dtype=np.float32))
    b_h = np.ascontiguousarray(np.asarray(b_h, dtype=np.float32))

    nc = _get_nc()
    in_maps = []
    for c in range(NCORES):
        s = slice(c * BL, (c + 1) * BL)
        in_maps.append(
            {
                "xT": np.ascontiguousarray(x[s].T),
                "hT": np.ascontiguousarray(h_prev[s].T),
                "att_score": np.ascontiguousarray(att_score[s]),
                "W_x": W_x,
                "b_x": b_x,
                "W_h": W_h,
                "b_h": b_h,
            }
        )
    res = run_bass_kernel_spmd(nc, in_maps, list(range(NCORES)))
    out = np.concatenate(
        [np.ascontiguousarray(res.results[c]["h_newT"].T) for c in range(NCORES)],
        axis=0,
    )
    return out


# revision 32
# speedup vs baseline: 1.1993x; 1.0096x over previous
"""AUGRU cell (attention-scaled GRU update) on 8 Trainium2 NeuronCores.

Data-parallel: batch B=65536 sharded 8 ways (8192 rows/core); gate weights
replicated.  Per core:

  gates_x = x @ W_x.T + b_x          (8192,384)
  gates_h = h @ W_h.T + b_h
  u = sigmoid(.. u block ..); r = sigmoid(.. r block ..)
  h_tilde = tanh(x_c + r * h_c)
  h_new = h_prev + att*u*(h_tilde - h_prev)

v8 design — gate-major layout, host-staged transposed operands:
  - each core receives xT/hT = x/h shard transposed to [I, rows] (a host
    layout/sharding choice; the contraction needs I on partitions either
    way) and the output is produced transposed, flipped back on the host.
  - gates live in PSUM as [gate_type][128, 512]: U/R/Cx/Ch banks.  Two
    accumulating fp32r matmuls for U and R, one each for Cx/Ch; weights
    transposed once at setup on the PE and kept fp32r (fp32-rate 1 cyc/row
    at N>=256, fp32-class accuracy).
  - biases are per-partition in this layout: sigmoid takes them via the
    ACT bias operand; the candidate path folds them into the two DVE
    scalar_tensor_tensor ops.  No bias matmuls, no device transposes,
    no PSUM round-trip copies, no casts on the matmul path.
  - epilogue: u/r/tanh outputs bf16; blend (t-h)*u*att in packed bf16 DVE
    ops; d and the final add against fp32 hT on GPSIMD keep h_prev exact.
"""

import sys

sys.path.insert(0, "/opt/trn_rl_repo")

import os
from contextlib import ExitStack

import numpy as np

import concourse.bass as bass
import concourse.tile as tile
from concourse import bacc, mybir
from concourse.bass_utils import run_bass_kernel_spmd
from concourse.masks import make_identity

F32 = mybir.dt.float32
F32R = mybir.dt.float32r
BF16 = mybir.dt.bfloat16
AF = mybir.ActivationFunctionType
OP = mybir.AluOpType

B = 65536
NCORES = 8
BL = int(os.environ.get("AUGRU_BL", B // NCORES))  # 8192 rows per core
I = 128
H = 128
G3 = 3 * H
P = 128
ROWS = 512  # batch rows per group (one fp32 PSUM bank per gate type)
NGROUPS = BL // ROWS

# PSUM banks per group: 0 = U, 1 = R, 2 = Cx, 3 = Ch   (each [128, 512])


def build_program():
    nc = bacc.Bacc("TRN2", target_bir_lowering=False, debug=False)

    xT_d = nc.dram_tensor("xT", [I, BL], F32, kind="ExternalInput").ap()
    hT_d = nc.dram_tensor("hT", [H, BL], F32, kind="ExternalInput").ap()
    a_d = nc.dram_tensor("att_score", [BL], F32, kind="ExternalInput").ap()
    wx_d = nc.dram_tensor("W_x", [G3, I], F32, kind="ExternalInput").ap()
    bx_d = nc.dram_tensor("b_x", [G3], F32, kind="ExternalInput").ap()
    wh_d = nc.dram_tensor("W_h", [G3, H], F32, kind="ExternalInput").ap()
    bh_d = nc.dram_tensor("b_h", [G3], F32, kind="ExternalInput").ap()
    o_d = nc.dram_tensor("h_newT", [H, BL], F32, kind="ExternalOutput").ap()

    with tile.TileContext(nc) as tc, ExitStack() as ctx:
        consts = ctx.enter_context(tc.tile_pool(name="consts", bufs=1))
        io = ctx.enter_context(tc.tile_pool(name="io", bufs=8))
        ep = ctx.enter_context(tc.tile_pool(name="ep", bufs=6))
        pg = ctx.enter_context(tc.tile_pool(name="pg", bufs=2, space="PSUM"))

        # ---------------- one-time setup ----------------
        ident = consts.tile([P, P], F32)
        make_identity(nc, ident)

        # weight blocks [128 gate, 128 I] -> transposed [128 I, 128 gate],
        # rounded to fp32r for full-rate fp32 matmuls
        wxn = consts.tile([P, 3, I], F32, tag="wxn")
        nc.sync.dma_start(wxn, wx_d.rearrange("(b g) i -> g b i", g=P))
        whn = consts.tile([P, 3, I], F32, tag="whn")
        nc.sync.dma_start(whn, wh_d.rearrange("(b g) i -> g b i", g=P))
        wT = consts.tile([P, 6, P], F32R, tag="wT")  # [xu, xr, xc, hu, hr, hc]
        for j in range(3):
            ps = pg.tile([P, ROWS], F32, tag="gU")
            nc.tensor.matmul(ps[:, 0:P], lhsT=wxn[:, j, :], rhs=ident,
                             is_transpose=True)
            nc.vector.tensor_copy(wT[:, j, :], ps[:, 0:P])
        for j in range(3):
            ps = pg.tile([P, ROWS], F32, tag="gU")
            nc.tensor.matmul(ps[:, 0:P], lhsT=whn[:, j, :], rhs=ident,
                             is_transpose=True)
            nc.vector.tensor_copy(wT[:, 3 + j, :], ps[:, 0:P])

        # per-partition bias columns [128, 1]: b_u+b_hu | b_r+b_hr | b_xc | b_hc
        bxc = consts.tile([P, 3], F32, tag="bxc")
        nc.sync.dma_start(bxc, bx_d.rearrange("(b p) -> p b", p=P))
        bhc = consts.tile([P, 3], F32, tag="bhc")
        nc.sync.dma_start(bhc, bh_d.rearrange("(b p) -> p b", p=P))
        bcol = consts.tile([P, 4], F32, tag="bcol")
        nc.vector.tensor_tensor(bcol[:, 0:2], bxc[:, 0:2], bhc[:, 0:2], OP.add)
        nc.vector.tensor_copy(bcol[:, 2:3], bxc[:, 2:3])
        nc.vector.tensor_copy(bcol[:, 3:4], bhc[:, 2:3])

        # att broadcast to all partitions (GPSIMD), then cast to bf16
        att1 = consts.tile([1, BL], F32R, tag="att1")
        nc.sync.dma_start(att1, a_d.unsqueeze(0).bitcast(F32R))
        ones_f = consts.tile([1, P], F32, tag="ones_f")
        nc.vector.memset(ones_f, 1.0)
        ones = consts.tile([1, P], F32R, tag="ones")
        nc.vector.tensor_copy(ones, ones_f)
        attb = consts.tile([P, BL], BF16, tag="attb")
        for gch in range(NGROUPS):
            ps = pg.tile([P, ROWS], F32, tag=("gR" if gch % 2 else "gCx"))
            nc.tensor.matmul(
                ps[:, :], lhsT=ones,
                rhs=att1[:, gch * ROWS : (gch + 1) * ROWS],
                start=True, stop=True,
            )
            dst = attb[:, gch * ROWS : (gch + 1) * ROWS]
            if gch % 2:
                nc.scalar.copy(dst, ps[:, :])
            else:
                nc.vector.tensor_copy(dst, ps[:, :])

        # ---------------- pipelined main loop ----------------
        stA = [None] * (NGROUPS + 4)
        stB = [None] * (NGROUPS + 4)
        stC = [None] * (NGROUPS + 4)

        def stage_a(g):
            b0 = g * ROWS
            xs = io.tile([P, ROWS], F32R, tag="xs")
            nc.sync.dma_start(xs, xT_d[:, b0 : b0 + ROWS].bitcast(F32R))
            hs = io.tile([P, ROWS], F32R, tag="hs")
            nc.sync.dma_start(hs, hT_d[:, b0 : b0 + ROWS].bitcast(F32R))
            return xs, hs

        def stage_b(g):
            xs, hs = stA[g]
            xr = xs
            hr = hs
            gU = pg.tile([P, ROWS], F32, tag="gU")
            gR = pg.tile([P, ROWS], F32, tag="gR")
            gCx = pg.tile([P, ROWS], F32, tag="gCx")
            gCh = pg.tile([P, ROWS], F32, tag="gCh")
            nc.tensor.matmul(gU, lhsT=wT[:, 0, :], rhs=xr, start=True, stop=False)
            nc.tensor.matmul(gR, lhsT=wT[:, 1, :], rhs=xr, start=True, stop=False)
            nc.tensor.matmul(gCx, lhsT=wT[:, 2, :], rhs=xr, start=True, stop=True)
            nc.tensor.matmul(gCh, lhsT=wT[:, 5, :], rhs=hr, start=True, stop=True)
            nc.tensor.matmul(gU, lhsT=wT[:, 3, :], rhs=hr, start=False, stop=True)
            nc.tensor.matmul(gR, lhsT=wT[:, 4, :], rhs=hr, start=False, stop=True)
            return gU, gR, gCx, gCh

        def stage_c(g):
            xs, hs = stA[g]
            gU, gR, gCx, gCh = stB[g]
            r = ep.tile([P, ROWS], BF16, tag="r")
            nc.scalar.activation(r, gR, AF.Sigmoid, bias=bcol[:, 1:2])
            u = ep.tile([P, ROWS], BF16, tag="u")
            nc.scalar.activation(u, gU, AF.Sigmoid, bias=bcol[:, 0:1])
            m = ep.tile([P, ROWS], F32, tag="m")
            nc.vector.scalar_tensor_tensor(
                m, in0=gCh, scalar=bcol[:, 3:4], in1=r,
                op0=OP.add, op1=OP.mult,
            )
            pre = ep.tile([P, ROWS], F32, tag="pre")
            nc.vector.scalar_tensor_tensor(
                pre, in0=gCx, scalar=bcol[:, 2:3], in1=m,
                op0=OP.add, op1=OP.add,
            )
            tb = ep.tile([P, ROWS], BF16, tag="tb")
            nc.scalar.activation(tb, pre, AF.Tanh)
            d = ep.tile([P, ROWS], BF16, tag="d")
            nc.gpsimd.tensor_tensor(d, tb, hs.bitcast(F32), OP.subtract)
            e1 = ep.tile([P, ROWS], BF16, tag="e1")
            nc.vector.tensor_tensor(e1, d, u, OP.mult)
            e2 = ep.tile([P, ROWS], BF16, tag="e2")
            nc.vector.tensor_tensor(
                e2, e1, attb[:, g * ROWS : (g + 1) * ROWS], OP.mult
            )
            return e2

        def stage_c2(g):
            b0 = g * ROWS
            xs, hs = stA[g]
            e2 = stC[g]
            ho = ep.tile([P, ROWS], F32, tag="ho")
            nc.gpsimd.tensor_tensor(ho, e2, hs.bitcast(F32), OP.add)
            # store from the GPSIMD queue: it directly follows ho there, so
            # the sync queue (loads) never head-of-line blocks on epilogues
            nc.gpsimd.dma_start(o_d[:, b0 : b0 + ROWS], ho)

        for k in range(NGROUPS + 4):
            if k < NGROUPS:
                stA[k] = stage_a(k)
            if 2 <= k < NGROUPS + 2:
                stB[k - 2] = stage_b(k - 2)
            if k >= 4:
                stage_c2(k - 4)
            if 3 <= k < NGROUPS + 3:
                stC[k - 3] = stage_c(k - 3)

    nc.compile()
    return nc


_NC_CACHE = []


def _get_nc():
    if not _NC_CACHE:
        _NC_CACHE.append(build_program())
    return _NC_CACHE[0]


def kernel(x, h_prev, att_score, W_x, b_x, W_h, b_h, **_unused):
    x = np.asarray(x, dtype=np.float32)
    h_prev = np.asarray(h_prev, dtype=np.float32)
    att_score = np.ascontiguousarray(np.asarray(att_score, dtype=np.float32))
    W_x = np.ascontiguousarray(np.asarray(W_x, dtype=np.float32))
    b_x = np.ascontiguousarray(np.asarray(b_x, dtype=np.float32))
    W_h = np.ascontiguousarray(np.asarray(W_h, dtype=np.float32))
    b_h = np.ascontiguousarray(np.asarray(b_h, dtype=np.float32))

    nc = _get_nc()
    in_maps = []
    for c in range(NCORES):
        s = slice(c * BL, (c + 1) * BL)
        in_maps.append(
            {
                "xT": np.ascontiguousarray(x[s].T),
                "hT": np.ascontiguousarray(h_prev[s].T),
                "att_score": np.ascontiguousarray(att_score[s]),
                "W_x": W_x,
                "b_x": b_x,
                "W_h": W_h,
                "b_h": b_h,
            }
        )
    res = run_bass_kernel_spmd(nc, in_maps, list(range(NCORES)))
    out = np.concatenate(
        [np.ascontiguousarray(res.results[c]["h_newT"].T) for c in range(NCORES)],
        axis=0,
    )
    return out


# revision 33
# speedup vs baseline: 1.2153x; 1.0134x over previous
"""AUGRU cell (attention-scaled GRU update) on 8 Trainium2 NeuronCores.

Data-parallel: batch B=65536 sharded 8 ways (8192 rows/core); gate weights
replicated.  Per core:

  gates_x = x @ W_x.T + b_x          (8192,384)
  gates_h = h @ W_h.T + b_h
  u = sigmoid(.. u block ..); r = sigmoid(.. r block ..)
  h_tilde = tanh(x_c + r * h_c)
  h_new = h_prev + att*u*(h_tilde - h_prev)

v8 design — gate-major layout, host-staged transposed operands:
  - each core receives xT/hT = x/h shard transposed to [I, rows] (a host
    layout/sharding choice; the contraction needs I on partitions either
    way) and the output is produced transposed, flipped back on the host.
  - gates live in PSUM as [gate_type][128, 512]: U/R/Cx/Ch banks.  Two
    accumulating fp32r matmuls for U and R, one each for Cx/Ch; weights
    transposed once at setup on the PE and kept fp32r (fp32-rate 1 cyc/row
    at N>=256, fp32-class accuracy).
  - biases are per-partition in this layout: sigmoid takes them via the
    ACT bias operand; the candidate path folds them into the two DVE
    scalar_tensor_tensor ops.  No bias matmuls, no device transposes,
    no PSUM round-trip copies, no casts on the matmul path.
  - epilogue: u/r/tanh outputs bf16; blend (t-h)*u*att in packed bf16 DVE
    ops; d and the final add against fp32 hT on GPSIMD keep h_prev exact.
"""

import sys

sys.path.insert(0, "/opt/trn_rl_repo")

import os
from contextlib import ExitStack

import numpy as np

import concourse.bass as bass
import concourse.tile as tile
from concourse import bacc, mybir
from concourse.bass_utils import run_bass_kernel_spmd

F32 = mybir.dt.float32
F32R = mybir.dt.float32r
BF16 = mybir.dt.bfloat16
AF = mybir.ActivationFunctionType
OP = mybir.AluOpType

B = 65536
NCORES = 8
BL = int(os.environ.get("AUGRU_BL", B // NCORES))  # 8192 rows per core
I = 128
H = 128
G3 = 3 * H
P = 128
ROWS = 512  # batch rows per group (one fp32 PSUM bank per gate type)
NGROUPS = BL // ROWS

# PSUM banks per group: 0 = U, 1 = R, 2 = Cx, 3 = Ch   (each [128, 512])


def build_program():
    nc = bacc.Bacc("TRN2", target_bir_lowering=False, debug=False)

    xT_d = nc.dram_tensor("xT", [I, BL], F32, kind="ExternalInput").ap()
    hT_d = nc.dram_tensor("hT", [H, BL], F32, kind="ExternalInput").ap()
    a_d = nc.dram_tensor("att_score", [BL], F32, kind="ExternalInput").ap()
    wx_d = nc.dram_tensor("wxT", [I, G3], F32, kind="ExternalInput").ap()
    bx_d = nc.dram_tensor("b_x", [G3], F32, kind="ExternalInput").ap()
    wh_d = nc.dram_tensor("whT", [H, G3], F32, kind="ExternalInput").ap()
    bh_d = nc.dram_tensor("b_h", [G3], F32, kind="ExternalInput").ap()
    o_d = nc.dram_tensor("h_newT", [H, BL], F32, kind="ExternalOutput").ap()

    with tile.TileContext(nc) as tc, ExitStack() as ctx:
        consts = ctx.enter_context(tc.tile_pool(name="consts", bufs=1))
        io = ctx.enter_context(tc.tile_pool(name="io", bufs=8))
        ep = ctx.enter_context(tc.tile_pool(name="ep", bufs=6))
        pg = ctx.enter_context(tc.tile_pool(name="pg", bufs=2, space="PSUM"))

        # ---------------- one-time setup ----------------
        # weights arrive host-transposed [I, 3*128]; DMA straight into the
        # fp32r stationary tile (no device transposes, no identity)
        wT = consts.tile([P, 6, P], F32R, tag="wT")  # [xu, xr, xc, hu, hr, hc]
        nc.sync.dma_start(
            wT[:, 0:3, :], wx_d.rearrange("i (b g) -> i b g", b=3).bitcast(F32R)
        )
        nc.sync.dma_start(
            wT[:, 3:6, :], wh_d.rearrange("i (b g) -> i b g", b=3).bitcast(F32R)
        )

        # per-partition bias columns [128, 1]: b_u+b_hu | b_r+b_hr | b_xc | b_hc
        bxc = consts.tile([P, 3], F32, tag="bxc")
        nc.sync.dma_start(bxc, bx_d.rearrange("(b p) -> p b", p=P))
        bhc = consts.tile([P, 3], F32, tag="bhc")
        nc.sync.dma_start(bhc, bh_d.rearrange("(b p) -> p b", p=P))
        bcol = consts.tile([P, 4], F32, tag="bcol")
        nc.vector.tensor_tensor(bcol[:, 0:2], bxc[:, 0:2], bhc[:, 0:2], OP.add)
        nc.vector.tensor_copy(bcol[:, 2:3], bxc[:, 2:3])
        nc.vector.tensor_copy(bcol[:, 3:4], bhc[:, 2:3])

        # att broadcast to all partitions (GPSIMD), then cast to bf16
        att1 = consts.tile([1, BL], F32R, tag="att1")
        nc.sync.dma_start(att1, a_d.unsqueeze(0).bitcast(F32R))
        ones_f = consts.tile([1, P], F32, tag="ones_f")
        nc.vector.memset(ones_f, 1.0)
        ones = consts.tile([1, P], F32R, tag="ones")
        nc.vector.tensor_copy(ones, ones_f)
        attb = consts.tile([P, BL], BF16, tag="attb")
        for gch in range(NGROUPS):
            ps = pg.tile([P, ROWS], F32, tag=("gR" if gch % 2 else "gCx"))
            nc.tensor.matmul(
                ps[:, :], lhsT=ones,
                rhs=att1[:, gch * ROWS : (gch + 1) * ROWS],
                start=True, stop=True,
            )
            dst = attb[:, gch * ROWS : (gch + 1) * ROWS]
            if gch % 2:
                nc.scalar.copy(dst, ps[:, :])
            else:
                nc.vector.tensor_copy(dst, ps[:, :])

        # ---------------- pipelined main loop ----------------
        stA = [None] * (NGROUPS + 4)
        stB = [None] * (NGROUPS + 4)
        stC = [None] * (NGROUPS + 4)

        def stage_a(g):
            b0 = g * ROWS
            xs = io.tile([P, ROWS], F32R, tag="xs")
            nc.sync.dma_start(xs, xT_d[:, b0 : b0 + ROWS].bitcast(F32R))
            hs = io.tile([P, ROWS], F32R, tag="hs")
            nc.sync.dma_start(hs, hT_d[:, b0 : b0 + ROWS].bitcast(F32R))
            return xs, hs

        def stage_b(g):
            xs, hs = stA[g]
            xr = xs
            hr = hs
            gU = pg.tile([P, ROWS], F32, tag="gU")
            gR = pg.tile([P, ROWS], F32, tag="gR")
            gCx = pg.tile([P, ROWS], F32, tag="gCx")
            gCh = pg.tile([P, ROWS], F32, tag="gCh")
            nc.tensor.matmul(gU, lhsT=wT[:, 0, :], rhs=xr, start=True, stop=False)
            nc.tensor.matmul(gR, lhsT=wT[:, 1, :], rhs=xr, start=True, stop=False)
            nc.tensor.matmul(gCx, lhsT=wT[:, 2, :], rhs=xr, start=True, stop=True)
            nc.tensor.matmul(gCh, lhsT=wT[:, 5, :], rhs=hr, start=True, stop=True)
            nc.tensor.matmul(gU, lhsT=wT[:, 3, :], rhs=hr, start=False, stop=True)
            nc.tensor.matmul(gR, lhsT=wT[:, 4, :], rhs=hr, start=False, stop=True)
            return gU, gR, gCx, gCh

        def stage_c(g):
            xs, hs = stA[g]
            gU, gR, gCx, gCh = stB[g]
            r = ep.tile([P, ROWS], BF16, tag="r")
            nc.scalar.activation(r, gR, AF.Sigmoid, bias=bcol[:, 1:2])
            u = ep.tile([P, ROWS], BF16, tag="u")
            nc.scalar.activation(u, gU, AF.Sigmoid, bias=bcol[:, 0:1])
            m = ep.tile([P, ROWS], F32, tag="m")
            nc.vector.scalar_tensor_tensor(
                m, in0=gCh, scalar=bcol[:, 3:4], in1=r,
                op0=OP.add, op1=OP.mult,
            )
            pre = ep.tile([P, ROWS], F32, tag="pre")
            nc.vector.scalar_tensor_tensor(
                pre, in0=gCx, scalar=bcol[:, 2:3], in1=m,
                op0=OP.add, op1=OP.add,
            )
            tb = ep.tile([P, ROWS], BF16, tag="tb")
            nc.scalar.activation(tb, pre, AF.Tanh)
            d = ep.tile([P, ROWS], BF16, tag="d")
            nc.gpsimd.tensor_tensor(d, tb, hs.bitcast(F32), OP.subtract)
            e1 = ep.tile([P, ROWS], BF16, tag="e1")
            nc.vector.tensor_tensor(e1, d, u, OP.mult)
            e2 = ep.tile([P, ROWS], BF16, tag="e2")
            nc.vector.tensor_tensor(
                e2, e1, attb[:, g * ROWS : (g + 1) * ROWS], OP.mult
            )
            return e2

        def stage_c2(g):
            b0 = g * ROWS
            xs, hs = stA[g]
            e2 = stC[g]
            ho = ep.tile([P, ROWS], F32, tag="ho")
            nc.gpsimd.tensor_tensor(ho, e2, hs.bitcast(F32), OP.add)
            # store from the GPSIMD queue: it directly follows ho there, so
            # the sync queue (loads) never head-of-line blocks on epilogues
            nc.gpsimd.dma_start(o_d[:, b0 : b0 + ROWS], ho)

        for k in range(NGROUPS + 4):
            if k < NGROUPS:
                stA[k] = stage_a(k)
            if 2 <= k < NGROUPS + 2:
                stB[k - 2] = stage_b(k - 2)
            if k >= 4:
                stage_c2(k - 4)
            if 3 <= k < NGROUPS + 3:
                stC[k - 3] = stage_c(k - 3)

    nc.compile()
    return nc


_NC_CACHE = []


def _get_nc():
    if not _NC_CACHE:
        _NC_CACHE.append(build_program())
    return _NC_CACHE[0]


def kernel(x, h_prev, att_score, W_x, b_x, W_h, b_h, **_unused):
    x = np.asarray(x, dtype=np.float32)
    h_prev = np.asarray(h_prev, dtype=np.float32)
    att_score = np.ascontiguousarray(np.asarray(att_score, dtype=np.float32))
    W_x = np.ascontiguousarray(np.asarray(W_x, dtype=np.float32))
    b_x = np.ascontiguousarray(np.asarray(b_x, dtype=np.float32))
    W_h = np.ascontiguousarray(np.asarray(W_h, dtype=np.float32))
    b_h = np.ascontiguousarray(np.asarray(b_h, dtype=np.float32))

    nc = _get_nc()
    in_maps = []
    for c in range(NCORES):
        s = slice(c * BL, (c + 1) * BL)
        in_maps.append(
            {
                "xT": np.ascontiguousarray(x[s].T),
                "hT": np.ascontiguousarray(h_prev[s].T),
                "att_score": np.ascontiguousarray(att_score[s]),
                "wxT": np.ascontiguousarray(W_x.T),
                "b_x": b_x,
                "whT": np.ascontiguousarray(W_h.T),
                "b_h": b_h,
            }
        )
    res = run_bass_kernel_spmd(nc, in_maps, list(range(NCORES)))
    out = np.concatenate(
        [np.ascontiguousarray(res.results[c]["h_newT"].T) for c in range(NCORES)],
        axis=0,
    )
    return out


# revision 34
# speedup vs baseline: 1.2793x; 1.0527x over previous
"""AUGRU cell (attention-scaled GRU update) on 8 Trainium2 NeuronCores.

Data-parallel: batch B=65536 sharded 8 ways (8192 rows/core); gate weights
replicated.  Per core:

  gates_x = x @ W_x.T + b_x          (8192,384)
  gates_h = h @ W_h.T + b_h
  u = sigmoid(.. u block ..); r = sigmoid(.. r block ..)
  h_tilde = tanh(x_c + r * h_c)
  h_new = h_prev + att*u*(h_tilde - h_prev)

v8 design — gate-major layout, host-staged transposed operands:
  - each core receives xT/hT = x/h shard transposed to [I, rows] (a host
    layout/sharding choice; the contraction needs I on partitions either
    way) and the output is produced transposed, flipped back on the host.
  - gates live in PSUM as [gate_type][128, 512]: U/R/Cx/Ch banks.  Two
    accumulating fp32r matmuls for U and R, one each for Cx/Ch; weights
    transposed once at setup on the PE and kept fp32r (fp32-rate 1 cyc/row
    at N>=256, fp32-class accuracy).
  - biases are per-partition in this layout: sigmoid takes them via the
    ACT bias operand; the candidate path folds them into the two DVE
    scalar_tensor_tensor ops.  No bias matmuls, no device transposes,
    no PSUM round-trip copies, no casts on the matmul path.
  - epilogue: u/r/tanh outputs bf16; blend (t-h)*u*att in packed bf16 DVE
    ops; d and the final add against fp32 hT on GPSIMD keep h_prev exact.
"""

import sys

sys.path.insert(0, "/opt/trn_rl_repo")

import os
from contextlib import ExitStack

import numpy as np

import concourse.bass as bass
import concourse.tile as tile
from concourse import bacc, mybir
from concourse.bass_utils import run_bass_kernel_spmd

F32 = mybir.dt.float32
F32R = mybir.dt.float32r
BF16 = mybir.dt.bfloat16
AF = mybir.ActivationFunctionType
OP = mybir.AluOpType

B = 65536
NCORES = 8
BL = int(os.environ.get("AUGRU_BL", B // NCORES))  # 8192 rows per core
I = 128
H = 128
G3 = 3 * H
P = 128
ROWS = 512  # batch rows per group (one fp32 PSUM bank per gate type)
NGROUPS = BL // ROWS

# PSUM banks per group: 0 = U, 1 = R, 2 = Cx, 3 = Ch   (each [128, 512])


def build_program():
    nc = bacc.Bacc("TRN2", target_bir_lowering=False, debug=False)

    xT_d = nc.dram_tensor("xT", [I, BL], F32, kind="ExternalInput").ap()
    hT_d = nc.dram_tensor("hT", [H, BL], F32, kind="ExternalInput").ap()
    a_d = nc.dram_tensor("att_score", [BL], F32, kind="ExternalInput").ap()
    wx_d = nc.dram_tensor("wxT", [I, G3], F32, kind="ExternalInput").ap()
    bx_d = nc.dram_tensor("b_x", [G3], F32, kind="ExternalInput").ap()
    wh_d = nc.dram_tensor("whT", [H, G3], F32, kind="ExternalInput").ap()
    bh_d = nc.dram_tensor("b_h", [G3], F32, kind="ExternalInput").ap()
    o_d = nc.dram_tensor("h_newT", [H, BL], F32, kind="ExternalOutput").ap()

    with tile.TileContext(nc) as tc, ExitStack() as ctx:
        consts = ctx.enter_context(tc.tile_pool(name="consts", bufs=1))
        io = ctx.enter_context(tc.tile_pool(name="io", bufs=8))
        ep = ctx.enter_context(tc.tile_pool(name="ep", bufs=6))
        pg = ctx.enter_context(tc.tile_pool(name="pg", bufs=2, space="PSUM"))

        # ---------------- one-time setup ----------------
        # weights arrive host-transposed [I, 3*128]; DMA straight into the
        # fp32r stationary tile (no device transposes, no identity)
        wT = consts.tile([P, 6, P], F32R, tag="wT")  # [xu, xr, xc, hu, hr, hc]
        nc.sync.dma_start(
            wT[:, 0:3, :], wx_d.rearrange("i (b g) -> i b g", b=3).bitcast(F32R)
        )
        nc.sync.dma_start(
            wT[:, 3:6, :], wh_d.rearrange("i (b g) -> i b g", b=3).bitcast(F32R)
        )

        # per-partition bias columns [128, 1]: b_u+b_hu | b_r+b_hr | b_xc | b_hc
        bxc = consts.tile([P, 3], F32, tag="bxc")
        nc.sync.dma_start(bxc, bx_d.rearrange("(b p) -> p b", p=P))
        bhc = consts.tile([P, 3], F32, tag="bhc")
        nc.sync.dma_start(bhc, bh_d.rearrange("(b p) -> p b", p=P))
        bcol = consts.tile([P, 4], F32, tag="bcol")
        nc.vector.tensor_tensor(bcol[:, 0:2], bxc[:, 0:2], bhc[:, 0:2], OP.add)
        nc.vector.tensor_copy(bcol[:, 2:3], bxc[:, 2:3])
        nc.vector.tensor_copy(bcol[:, 3:4], bhc[:, 2:3])

        # att broadcast to all partitions (GPSIMD), then cast to bf16
        att1 = consts.tile([1, BL], F32R, tag="att1")
        nc.sync.dma_start(att1, a_d.unsqueeze(0).bitcast(F32R))
        ones_f = consts.tile([1, P], F32, tag="ones_f")
        nc.vector.memset(ones_f, 1.0)
        ones = consts.tile([1, P], F32R, tag="ones")
        nc.vector.tensor_copy(ones, ones_f)
        attb = consts.tile([P, BL], BF16, tag="attb")

        # ---------------- pipelined main loop ----------------
        stA = [None] * (NGROUPS + 4)
        stB = [None] * (NGROUPS + 4)
        stC = [None] * (NGROUPS + 4)

        def stage_a(g):
            b0 = g * ROWS
            xs = io.tile([P, ROWS], F32R, tag="xs")
            nc.sync.dma_start(xs, xT_d[:, b0 : b0 + ROWS].bitcast(F32R))
            hs = io.tile([P, ROWS], F32R, tag="hs")
            nc.sync.dma_start(hs, hT_d[:, b0 : b0 + ROWS].bitcast(F32R))
            # att broadcast chunk for this group: a K=1 matmul that fills PE
            # idle during the pipeline ramp, well ahead of e2(g)'s need
            ps = pg.tile([P, ROWS], F32, tag=("gR" if g % 2 else "gCx"))
            nc.tensor.matmul(
                ps[:, :], lhsT=ones,
                rhs=att1[:, b0 : b0 + ROWS],
                start=True, stop=True,
            )
            dst = attb[:, b0 : b0 + ROWS]
            if g % 2:
                nc.scalar.copy(dst, ps[:, :])
            else:
                nc.vector.tensor_copy(dst, ps[:, :])
            return xs, hs

        def stage_b(g):
            xs, hs = stA[g]
            xr = xs
            hr = hs
            gU = pg.tile([P, ROWS], F32, tag="gU")
            gR = pg.tile([P, ROWS], F32, tag="gR")
            gCx = pg.tile([P, ROWS], F32, tag="gCx")
            gCh = pg.tile([P, ROWS], F32, tag="gCh")
            nc.tensor.matmul(gU, lhsT=wT[:, 0, :], rhs=xr, start=True, stop=False)
            nc.tensor.matmul(gR, lhsT=wT[:, 1, :], rhs=xr, start=True, stop=False)
            nc.tensor.matmul(gCx, lhsT=wT[:, 2, :], rhs=xr, start=True, stop=True)
            nc.tensor.matmul(gCh, lhsT=wT[:, 5, :], rhs=hr, start=True, stop=True)
            nc.tensor.matmul(gU, lhsT=wT[:, 3, :], rhs=hr, start=False, stop=True)
            nc.tensor.matmul(gR, lhsT=wT[:, 4, :], rhs=hr, start=False, stop=True)
            return gU, gR, gCx, gCh

        def stage_c(g):
            xs, hs = stA[g]
            gU, gR, gCx, gCh = stB[g]
            r = ep.tile([P, ROWS], BF16, tag="r")
            nc.scalar.activation(r, gR, AF.Sigmoid, bias=bcol[:, 1:2])
            u = ep.tile([P, ROWS], BF16, tag="u")
            nc.scalar.activation(u, gU, AF.Sigmoid, bias=bcol[:, 0:1])
            m = ep.tile([P, ROWS], F32, tag="m")
            nc.vector.scalar_tensor_tensor(
                m, in0=gCh, scalar=bcol[:, 3:4], in1=r,
                op0=OP.add, op1=OP.mult,
            )
            pre = ep.tile([P, ROWS], F32, tag="pre")
            nc.vector.scalar_tensor_tensor(
                pre, in0=gCx, scalar=bcol[:, 2:3], in1=m,
                op0=OP.add, op1=OP.add,
            )
            tb = ep.tile([P, ROWS], BF16, tag="tb")
            nc.scalar.activation(tb, pre, AF.Tanh)
            d = ep.tile([P, ROWS], BF16, tag="d")
            nc.gpsimd.tensor_tensor(d, tb, hs.bitcast(F32), OP.subtract)
            e1 = ep.tile([P, ROWS], BF16, tag="e1")
            nc.vector.tensor_tensor(e1, d, u, OP.mult)
            e2 = ep.tile([P, ROWS], BF16, tag="e2")
            nc.vector.tensor_tensor(
                e2, e1, attb[:, g * ROWS : (g + 1) * ROWS], OP.mult
            )
            return e2

        def stage_c2(g):
            b0 = g * ROWS
            xs, hs = stA[g]
            e2 = stC[g]
            ho = ep.tile([P, ROWS], F32, tag="ho")
            nc.gpsimd.tensor_tensor(ho, e2, hs.bitcast(F32), OP.add)
            # store from the GPSIMD queue: it directly follows ho there, so
            # the sync queue (loads) never head-of-line blocks on epilogues
            nc.gpsimd.dma_start(o_d[:, b0 : b0 + ROWS], ho)

        for k in range(NGROUPS + 4):
            if k < NGROUPS:
                stA[k] = stage_a(k)
            if 2 <= k < NGROUPS + 2:
                stB[k - 2] = stage_b(k - 2)
            if k >= 4:
                stage_c2(k - 4)
            if 3 <= k < NGROUPS + 3:
                stC[k - 3] = stage_c(k - 3)

    nc.compile()
    return nc


_NC_CACHE = []


def _get_nc():
    if not _NC_CACHE:
        _NC_CACHE.append(build_program())
    return _NC_CACHE[0]


def kernel(x, h_prev, att_score, W_x, b_x, W_h, b_h, **_unused):
    x = np.asarray(x, dtype=np.float32)
    h_prev = np.asarray(h_prev, dtype=np.float32)
    att_score = np.ascontiguousarray(np.asarray(att_score, dtype=np.float32))
    W_x = np.ascontiguousarray(np.asarray(W_x, dtype=np.float32))
    b_x = np.ascontiguousarray(np.asarray(b_x, dtype=np.float32))
    W_h = np.ascontiguousarray(np.asarray(W_h, dtype=np.float32))
    b_h = np.ascontiguousarray(np.asarray(b_h, dtype=np.float32))

    nc = _get_nc()
    in_maps = []
    for c in range(NCORES):
        s = slice(c * BL, (c + 1) * BL)
        in_maps.append(
            {
                "xT": np.ascontiguousarray(x[s].T),
                "hT": np.ascontiguousarray(h_prev[s].T),
                "att_score": np.ascontiguousarray(att_score[s]),
                "wxT": np.ascontiguousarray(W_x.T),
                "b_x": b_x,
                "whT": np.ascontiguousarray(W_h.T),
                "b_h": b_h,
            }
        )
    res = run_bass_kernel_spmd(nc, in_maps, list(range(NCORES)))
    out = np.concatenate(
        [np.ascontiguousarray(res.results[c]["h_newT"].T) for c in range(NCORES)],
        axis=0,
    )
    return out
